# revision 16
# baseline (speedup 1.0000x reference)
"""Trainium2 Bass kernel for gpt-oss AttentionBlock (full causal + sinks).

Sharding: head-parallel across 8 cores. Core c owns KV head c and query heads
{g*8+c, g=0..7} (GQA mapping h = g*8 + kv), plus their sink logits. Each core
computes the QKV projection (rnorm folded into x on host), RoPE, causal
attention with sink in the softmax denominator, and a partial out-projection
y_c = o_c @ Wo_c^T. Host sums the 8 bf16 partials + out_b + residual x.

v2 notes (vs v1):
- bf16 everywhere off-chip and for matmul operands (validated ~7e-3 rel err
  vs the 2e-2 gate); fp32 only in PSUM accumulation and the softmax
  denominator reciprocal.
- rnorm pre-folded into xT columns on host (kills the ACT scale pass).
- PSUM budget: score tiles are per-head [128,512] (1 bank, bufs=3), PV
  accumulators [65,512] double-buffered (2+2 banks), outproj 1 bank. PV
  double-buffering removes the per-gp PE stall that kept HAM cold.
- reciprocal_approx_fast (~5x faster than reciprocal; 18 bits is plenty for
  bf16 probs), batched per-gp over both heads.
- wo prefetched on the scalar HWDGE queue during phase 1.
- softmax without max-subtraction: scores bounded, so
  attn = exp(s)/(sum exp(s) + exp(sink)) (== reference algebraically).
- scores computed transposed [sk, sq] so P^T feeds PV directly; PV output
  o^T [d, sq] feeds the out-proj directly. No S x S transposes anywhere.
"""

import math
import os
import sys
from collections import deque

sys.path.insert(0, "/opt/trn_rl_repo")

import numpy as np
import ml_dtypes

BF16 = ml_dtypes.bfloat16

# ---- problem constants (hardcoded per contract) ----
HID = 2880
S = 1536
N_HEADS = 64
N_KV = 8
D = 64
G = 8
SM_SCALE = 1.0 / math.sqrt(D)
EPS = 1e-5
NCORES = 8

ROPE_BASE = 150000.0
INIT_CTX = 4096
SCALING = 32.0
NTK_ALPHA = 1.0
NTK_BETA = 32.0

KP = 2944          # padded contraction dim: 2880 + bias row + zero pad = 23*128
KCH = KP // 128    # 23
QKV_O = 640        # 512 q + 64 k + 64 v per core
ETILES = (HID + 127) // 128  # 23 (22*128 + 64)


def _rope_tables(num_tokens: int):
    d_half = D // 2
    freq = ROPE_BASE ** (np.arange(0, D, 2, dtype=np.float64) / D)
    concentration = 0.1 * math.log(SCALING) + 1.0
    low = d_half * math.log(INIT_CTX / (NTK_BETA * 2 * math.pi)) / math.log(ROPE_BASE)
    high = d_half * math.log(INIT_CTX / (NTK_ALPHA * 2 * math.pi)) / math.log(ROPE_BASE)
    interpolation = 1.0 / (SCALING * freq)
    extrapolation = 1.0 / freq
    ramp = (np.arange(d_half, dtype=np.float64) - low) / (high - low)
    mask = 1.0 - np.clip(ramp, 0.0, 1.0)
    inv_freq = interpolation * (1.0 - mask) + extrapolation * mask
    t = np.arange(num_tokens, dtype=np.float64)
    freqs = np.outer(t, inv_freq)
    cos = (np.cos(freqs) * concentration).astype(np.float32)
    sin = (np.sin(freqs) * concentration).astype(np.float32)
    return cos, sin


_PROGRAM = None
LAST_EXEC_NS = None
LAST_RESULTS = None


def _build_program(s_len=S, reps=1):
    import concourse.bacc as bacc
    import concourse.tile as tile
    from concourse import mybir
    from contextlib import ExitStack

    f32 = mybir.dt.float32
    bf = mybir.dt.bfloat16
    Act = mybir.ActivationFunctionType

    stiles = s_len // 128
    sqc = s_len // 512

    nc = bacc.Bacc("TRN2", target_bir_lowering=False, debug=False)

    xT = nc.dram_tensor("xT", [KP, s_len], bf, kind="ExternalInput")
    wqkv = nc.dram_tensor("wqkv", [KP, QKV_O], bf, kind="ExternalInput")
    wo = nc.dram_tensor("wo", [512, HID], bf, kind="ExternalInput")
    cosd = nc.dram_tensor("cosd", [128, stiles * 32], bf, kind="ExternalInput")
    sind = nc.dram_tensor("sind", [128, stiles * 32], bf, kind="ExternalInput")
    sinkw = nc.dram_tensor("sinkw", [1, 8 * 65], bf, kind="ExternalInput")
    idend = nc.dram_tensor("idend", [128, 128], bf, kind="ExternalInput")
    maskd = nc.dram_tensor("maskd", [128, 4 * 512], bf, kind="ExternalInput")
    yT = nc.dram_tensor("yT", [HID, s_len], bf, kind="ExternalOutput")

    xT_r = xT[:].rearrange("(j p) s -> p j s", p=128)
    wqkv_r = wqkv[:].rearrange("(j p) o -> p j o", p=128)
    wo_r = wo[:].rearrange("(b p) e -> p b e", p=128)

    import concourse.bass as bass

    def bcast_mid(ap2d, n):
        a = ap2d
        return bass.AP(a.tensor, a.offset, [list(a.ap[0]), [0, n], list(a.ap[1])])

    with ExitStack() as top:
        tc = top.enter_context(tile.TileContext(nc))
        consts = top.enter_context(tc.tile_pool(name="consts", bufs=1))
        persist = top.enter_context(tc.tile_pool(name="persist", bufs=1))

        iden = consts.tile([128, 128], bf)
        nc.gpsimd.dma_start(out=iden[:], in_=idend[:])
        negm = consts.tile([128, 4, 512], bf)
        nc.gpsimd.dma_start(out=negm[:], in_=maskd[:].rearrange("p (l f) -> p l f", l=4))
        cos_t = consts.tile([128, stiles, 32], bf)
        nc.gpsimd.dma_start(out=cos_t[:], in_=cosd[:].rearrange("p (i d) -> p i d", d=32))
        sin_t = consts.tile([128, stiles, 32], bf)
        nc.gpsimd.dma_start(out=sin_t[:], in_=sind[:].rearrange("p (i d) -> p i d", d=32))
        sink_t = consts.tile([1, 8, 65], bf)
        nc.gpsimd.dma_start(out=sink_t[:], in_=sinkw[:].rearrange("p (g o) -> p g o", g=8))
        ones_row = consts.tile([1, 512], bf)
        nc.vector.memset(ones_row[:], 1.0)
        # dummy partition_broadcast: preloads the Q7 custom-op library during
        # phase 1 so the first softmax epilogue doesn't eat the LOAD_LIB stall
        gpw_in = consts.tile([1, 512], f32)
        nc.vector.memset(gpw_in[:], 1.0)
        gpw_out = consts.tile([64, 512], f32)
        nc.gpsimd.partition_broadcast(gpw_out[:], gpw_in[:], channels=64)

        qT2 = persist.tile([128, 8, s_len], bf)   # rows 0:64 = qT, 64:128 = dup
        kT2 = persist.tile([128, s_len], bf)
        vaug = persist.tile([128, stiles, 65], bf)
        nc.vector.memset(vaug[:, :, 64:65], 1.0)
        oT_s = persist.tile([128, 4, s_len], bf)
        wo_t = persist.tile([128, 4, HID], bf)

        for _rep in range(reps):
          # ---------------- phase 1-3: qkv proj + rope + transposes --------------
          with ExitStack() as ph1:
              wqp = ph1.enter_context(tc.tile_pool(name="wq", bufs=1))
              xsp = ph1.enter_context(tc.tile_pool(name="xs", bufs=2))
              qkvp = ph1.enter_context(tc.tile_pool(name="qkv", bufs=4))
              ropp = ph1.enter_context(tc.tile_pool(name="rop", bufs=4))
              tmpp = ph1.enter_context(tc.tile_pool(name="rtmp", bufs=3))
              p1a = ph1.enter_context(tc.tile_pool(name="p1a", bufs=2, space="PSUM"))
              p1b = ph1.enter_context(tc.tile_pool(name="p1b", bufs=2, space="PSUM"))
              ptr = ph1.enter_context(tc.tile_pool(name="ptr", bufs=3, space="PSUM"))

              # prefetch wo on the scalar HWDGE queue (parallel to sync queue)
              nc.scalar.dma_start(out=wo_t[:, :, 0:1440], in_=wo_r[:, :, 0:1440])
              nc.scalar.dma_start(out=wo_t[:, :, 1440:HID], in_=wo_r[:, :, 1440:HID])

              # first weight chunk, then first x block, then remaining weights
              wq_t = wqp.tile([128, KCH, QKV_O], bf)
              nc.sync.dma_start(out=wq_t[:, 0:1, :], in_=wqkv_r[:, 0:1, :])
              xt0 = xsp.tile([128, KCH, 256], bf, tag="xt")
              nc.sync.dma_start(out=xt0[:], in_=xT_r[:, :, 0:256])
              for a, b in ((1, 2), (2, 4), (4, 6), (6, 9), (9, 12),
                           (12, 16), (16, 20), (20, KCH)):
                  nc.sync.dma_start(out=wq_t[:, a:b, :], in_=wqkv_r[:, a:b, :])

              xt = xt0
              for i in range(stiles):
                  i2, sb = divmod(i, 2)
                  if sb == 0 and i2 > 0:
                      xt = xsp.tile([128, KCH, 256], bf, tag="xt")
                      nc.sync.dma_start(out=xt[:], in_=xT_r[:, :, i2 * 256:(i2 + 1) * 256])
                  xs2 = xt[:, :, sb * 128:(sb + 1) * 128]
                  pa = p1a.tile([128, 384], f32)
                  pb = p1b.tile([128, 256], f32)
                  for j in range(KCH):
                      nc.tensor.matmul(pa[:], xs2[:, j, :], wq_t[:, j, 0:384],
                                       start=(j == 0), stop=(j == KCH - 1))
                      nc.tensor.matmul(pb[:], xs2[:, j, :], wq_t[:, j, 384:640],
                                       start=(j == 0), stop=(j == KCH - 1))
                  qkvt = qkvp.tile([128, QKV_O], bf)
                  nc.scalar.activation(qkvt[:, 0:384], pa[:], Act.Copy)
                  nc.scalar.activation(qkvt[:, 384:640], pb[:], Act.Copy)

                  ro = ropp.tile([128, 576], bf)
                  # one strided view covers q heads 0-7 (stride 64) AND k (at 512)
                  q3 = qkvt[:, 0:576].rearrange("p (h d) -> p h d", h=9)
                  r3 = ro[:, 0:576].rearrange("p (h d) -> p h d", h=9)
                  ctb = bcast_mid(cos_t[:, i, :], 9)
                  stb = bcast_mid(sin_t[:, i, :], 9)
                  t1 = tmpp.tile([128, 9, 32], bf, tag="t1")
                  t2 = tmpp.tile([128, 9, 32], bf, tag="t2")
                  t3 = tmpp.tile([128, 9, 32], bf, tag="t3")
                  t4 = tmpp.tile([128, 9, 32], bf, tag="t4")
                  x1 = q3[:, :, 0:32]
                  x2 = q3[:, :, 32:64]
                  nc.vector.tensor_mul(t1[:], x1, ctb)
                  nc.vector.tensor_mul(t2[:], x2, stb)
                  nc.vector.tensor_sub(r3[:, :, 0:32], t1[:], t2[:])
                  nc.vector.tensor_mul(t3[:], x2, ctb)
                  nc.vector.tensor_mul(t4[:], x1, stb)
                  nc.vector.tensor_add(r3[:, :, 32:64], t3[:], t4[:])

                  nc.vector.tensor_copy(vaug[:, i, 0:64], qkvt[:, 576:640])

                  for gq in range(2):
                      ptq = ptr.tile([64, 512], bf, tag="pt")
                      for g4 in range(4):
                          g = gq * 4 + g4
                          nc.tensor.transpose(ptq[:, g4 * 128:(g4 + 1) * 128],
                                              ro[:, g * 64:(g + 1) * 64], iden[:])
                      nc.vector.tensor_copy(
                          qT2[0:64, gq * 4:(gq + 1) * 4, i * 128:(i + 1) * 128],
                          ptq[:].rearrange("p (h f) -> p h f", h=4))
                  ptk = ptr.tile([64, 512], bf, tag="pt")
                  nc.tensor.transpose(ptk[:, 0:128], ro[:, 512:576], iden[:])
                  nc.vector.tensor_copy(kT2[0:64, i * 128:(i + 1) * 128], ptk[:, 0:128])

                  if i % 4 == 3:
                      jc = i // 4
                      sq = slice(jc * 512, (jc + 1) * 512)
                      # duplicate q/k rows into partitions 64:128 for row-group packing
                      nc.sync.dma_start(out=qT2[64:128, :, sq], in_=qT2[0:64, :, sq])
                      nc.sync.dma_start(out=kT2[64:128, sq], in_=kT2[0:64, sq])

          # ---------------- phase 4+5: attention + out proj, interleaved ---------
          with ExitStack() as ph45:
              ptp = ph45.enter_context(tc.tile_pool(name="ptile", bufs=8))
              nrm = ph45.enter_context(tc.tile_pool(name="nrm", bufs=2))
              rbp = ph45.enter_context(tc.tile_pool(name="rbp", bufs=4))
              ytsp = ph45.enter_context(tc.tile_pool(name="yts", bufs=6))

              def make_emit(pool, nbufs):
                  def emit_outproj_block(jc_src, et, tail=False):
                      sqo = slice(jc_src * 512, (jc_src + 1) * 512)
                      esz = min(128, HID - et * 128)
                      es = slice(et * 128, et * 128 + esz)
                      ytp = pool.tile([128, 512], f32, tag="ytp", bufs=nbufs)
                      for b in range(4):
                          nc.tensor.matmul(ytp[0:esz, :], wo_t[:, b, es],
                                           oT_s[:, b, sqo],
                                           start=(b == 0), stop=(b == 3))
                      yts = ytsp.tile([128, 512], bf, tag="yts")
                      if tail:
                          nc.scalar.activation(yts[0:esz, :], ytp[0:esz, :], Act.Copy)
                      else:
                          nc.vector.tensor_copy(yts[0:esz, :], ytp[0:esz, :])
                      nc.sync.dma_start(out=yT[es, sqo], in_=yts[0:esz, :])
                  return emit_outproj_block

              pend = deque()
              attn = ExitStack()
              scp = attn.enter_context(tc.tile_pool(name="sc", bufs=3, space="PSUM"))
              pvp = attn.enter_context(tc.tile_pool(name="pv", bufs=2, space="PSUM"))
              ytpp = attn.enter_context(tc.tile_pool(name="ytp", bufs=1, space="PSUM"))
              emit_outproj_block = make_emit(ytpp, 1)
              # dense dummy matmuls in the phase-transition stall: flips the
              # HAM clock gate to 8/8 before the ACT-paced attention stream
              # (which alone has too low a PE duty cycle to warm it)
              for wi in range(16):
                  scw = scp.tile([128, 512], f32, tag="sc")
                  nc.tensor.matmul(scw[:], kT2[0:64, 0:128], qT2[0:64, 0, 0:512],
                                   start=True, stop=True, tile_position=(0, 0))
              for jc in range(sqc):
                  sq = slice(jc * 512, (jc + 1) * 512)
                  nsk = 4 * (jc + 1)
                  for gp in range(4):
                      g0, g1 = 2 * gp, 2 * gp + 1
                      pva = pvp.tile([65, 512], f32, tag="pva")
                      pvb = pvp.tile([65, 512], f32, tag="pvb")
                      nc.tensor.matmul(pva[:], sink_t[:, g0, :], ones_row[:],
                                       start=True, stop=False)
                      nc.tensor.matmul(pvb[:], sink_t[:, g1, :], ones_row[:],
                                       start=True, stop=False)
                      for isk in range(nsk):
                          ks = slice(isk * 128, (isk + 1) * 128)
                          lsi = isk - 4 * jc
                          sca = scp.tile([128, 512], f32, tag="sc")
                          scb = scp.tile([128, 512], f32, tag="sc")
                          nc.tensor.matmul(sca[:], kT2[0:64, ks], qT2[0:64, g0, sq],
                                           start=True, stop=True,
                                           tile_position=(0, 0))
                          nc.tensor.matmul(scb[:], kT2[64:128, ks], qT2[64:128, g1, sq],
                                           start=True, stop=True,
                                           tile_position=(64, 0))
                          pta = ptp.tile([128, 512], bf, tag="pt")
                          ptb = ptp.tile([128, 512], bf, tag="pt")
                          nc.scalar.activation(pta[:], sca[:], Act.Exp, scale=SM_SCALE)
                          nc.scalar.activation(ptb[:], scb[:], Act.Exp, scale=SM_SCALE)
                          if lsi >= 0:
                              w = 128 * (lsi + 1)  # mask is all-ones beyond col w
                              nc.vector.tensor_mul(pta[:, 0:w], pta[:, 0:w],
                                                   negm[:, lsi, 0:w])
                              nc.vector.tensor_mul(ptb[:, 0:w], ptb[:, 0:w],
                                                   negm[:, lsi, 0:w])
                          nc.tensor.matmul(pva[:], vaug[:, isk, :], pta[:],
                                           start=False, stop=(isk == nsk - 1))
                          nc.tensor.matmul(pvb[:], vaug[:, isk, :], ptb[:],
                                           start=False, stop=(isk == nsk - 1))
                          if pend:
                              emit_outproj_block(*pend.popleft())
                          elif jc == 0:
                              # no out-proj work yet: dense filler matmul so
                              # the HAM clock gate stays at 8/8 through jc=0
                              scw = scp.tile([128, 512], f32, tag="sc")
                              nc.tensor.matmul(scw[:], kT2[0:64, 0:128],
                                               qT2[0:64, 0, 0:512],
                                               start=True, stop=True,
                                               tile_position=(0, 0))
                      # softmax epilogue: one reciprocal covers both heads
                      # (denoms staged at 32-aligned partitions; recip cost
                      # depends on free-size only, so [33,512] == [1,512])
                      dn = nrm.tile([33, 512], f32, tag="dn")
                      nc.vector.tensor_copy(dn[0:1, :], pva[64:65, :])
                      nc.vector.tensor_copy(dn[32:33, :], pvb[64:65, :])
                      rec = nrm.tile([33, 512], f32, tag="rec")
                      nc.vector.reciprocal(rec[:], dn[:])
                      # partition_broadcast (Q7 custom op) needs base-0 input
                      recb0 = nrm.tile([1, 512], f32, tag="recb0")
                      nc.vector.tensor_copy(recb0[:], rec[32:33, :])
                      rba = rbp.tile([64, 512], f32, tag="rb")
                      rbb = rbp.tile([64, 512], f32, tag="rb")
                      nc.gpsimd.partition_broadcast(rba[:], rec[0:1, :], channels=64)
                      nc.gpsimd.partition_broadcast(rbb[:], recb0[:], channels=64)
                      nc.vector.tensor_mul(oT_s[0:64, gp, sq], pva[0:64, :], rba[:])
                      # partition-shifted DVE writes are sim-only; stage + DMA
                      ot = rbp.tile([64, 512], bf, tag="ot")
                      nc.vector.tensor_mul(ot[:], pvb[0:64, :], rbb[:])
                      nc.sync.dma_start(out=oT_s[64:128, gp, sq], in_=ot[:])
                      if pend and jc < sqc - 1:
                          emit_outproj_block(*pend.popleft())
                  pend.extend((jc, et) for et in range(ETILES))
              # tail drain: free the attention PSUM banks and run the last
              # out-proj blocks through a deep 4-bank ring so they pipeline
              attn.close()
              with tc.tile_pool(name="ytail", bufs=4, space="PSUM") as ytailp:
                  emit_tail = make_emit(ytailp, 4)
                  while pend:
                      emit_tail(*pend.popleft(), tail=True)

    nc.finalize()
    return nc


def _get_program():
    global _PROGRAM
    if _PROGRAM is None:
        _PROGRAM = _build_program(S)
    return _PROGRAM


def _host_inputs(x, sinks, norm_scale, qkv_w, qkv_b, out_w, s_len=S):
    xf = np.ascontiguousarray(np.asarray(x, np.float32).reshape(s_len, HID))
    ms = np.mean(xf * xf, axis=1, dtype=np.float32)
    rnorm = (1.0 / np.sqrt(ms + np.float32(EPS))).astype(np.float32)
    cos, sin = _rope_tables(s_len)
    stiles = s_len // 128

    xTp = np.zeros((KP, s_len), BF16)
    xTp[:HID] = (xf.T * rnorm[None, :]).astype(BF16)
    xTp[HID] = BF16(1.0)  # bias row

    nsc = np.asarray(norm_scale, np.float32)
    qkvw = np.asarray(qkv_w, np.float32) * nsc[None, :]
    qkvb = np.asarray(qkv_b, np.float32)
    ow = np.asarray(out_w, np.float32)
    sk = np.asarray(sinks, np.float32)

    cos_t = np.ascontiguousarray(
        cos.reshape(stiles, 128, 32).transpose(1, 0, 2).reshape(128, stiles * 32)
    ).astype(BF16)
    sin_t = np.ascontiguousarray(
        sin.reshape(stiles, 128, 32).transpose(1, 0, 2).reshape(128, stiles * 32)
    ).astype(BF16)
    iden = np.eye(128, dtype=BF16)
    # msk[l][p,f] = 1 if valid (f >= 128*l + p) else 0; multiplies exp output
    pp = np.arange(128)[:, None]
    ff = np.arange(512)[None, :]
    masks = np.stack([(ff >= 128 * l + pp).astype(BF16) for l in range(4)], 1)
    masks = np.ascontiguousarray(masks.reshape(128, 4 * 512))

    in_maps = []
    for c in range(NCORES):
        heads = [g * 8 + c for g in range(G)]
        wq = np.concatenate([qkvw[h * 64:(h + 1) * 64] for h in heads], 0)
        wk = qkvw[4096 + c * 64:4096 + (c + 1) * 64]
        wv = qkvw[4608 + c * 64:4608 + (c + 1) * 64]
        wqkv_c = np.concatenate([wq, wk, wv], 0)          # [640, 2880]
        bq = np.concatenate([qkvb[h * 64:(h + 1) * 64] for h in heads]
                            + [qkvb[4096 + c * 64:4096 + (c + 1) * 64],
                               qkvb[4608 + c * 64:4608 + (c + 1) * 64]])
        wq_pad = np.zeros((KP, QKV_O), BF16)
        wq_pad[:HID] = wqkv_c.T.astype(BF16)
        wq_pad[HID] = bq.astype(BF16)
        cols = np.concatenate([np.arange(h * 64, (h + 1) * 64) for h in heads])
        woT = np.ascontiguousarray(ow[:, cols].T).astype(BF16)  # [512, 2880]
        sinkw = np.zeros((8, 65), BF16)
        for g in range(G):
            sinkw[g, 64] = BF16(np.exp(sk[heads[g]]))
        in_maps.append({
            "xT": xTp, "wqkv": wq_pad, "wo": woT,
            "cosd": cos_t, "sind": sin_t,
            "sinkw": sinkw.reshape(1, 8 * 65), "idend": iden, "maskd": masks,
        })
    return in_maps, xf


def kernel(x, sinks, norm_scale, qkv_w, qkv_b, out_w, out_b):
    global LAST_EXEC_NS, LAST_RESULTS
    from concourse.bass_utils import run_bass_kernel_spmd

    B = x.shape[0]
    in_maps, xf = _host_inputs(x, sinks, norm_scale, qkv_w, qkv_b, out_w)
    nc = _get_program()
    trace = bool(os.environ.get("KERNEL_TRACE"))
    if trace:
        try:
            from antenv.axon_hooks import get_axon_ntff_profile_hook  # noqa: F401
        except Exception:
            trace = False
    r = run_bass_kernel_spmd(nc, in_maps, core_ids=list(range(NCORES)), trace=trace)
    LAST_EXEC_NS = r.exec_time_ns
    LAST_RESULTS = r
    y = np.zeros((S, HID), np.float32)
    for c in range(NCORES):
        y += r.results[c]["yT"].T.astype(np.float32)
    out = xf + y + np.asarray(out_b, np.float32)[None, :]
    return out.reshape(B, S, HID).astype(np.float32)


# revision 17
# speedup vs baseline: 1.0319x; 1.0319x over previous
"""Trainium2 Bass kernel for gpt-oss AttentionBlock (full causal + sinks).

Sharding: head-parallel across 8 cores. Core c owns KV head c and query heads
{g*8+c, g=0..7} (GQA mapping h = g*8 + kv), plus their sink logits. Each core
computes the QKV projection (rnorm folded into x on host), RoPE, causal
attention with sink in the softmax denominator, and a partial out-projection
y_c = o_c @ Wo_c^T. Host sums the 8 bf16 partials + out_b + residual x.

v2 notes (vs v1):
- bf16 everywhere off-chip and for matmul operands (validated ~7e-3 rel err
  vs the 2e-2 gate); fp32 only in PSUM accumulation and the softmax
  denominator reciprocal.
- rnorm pre-folded into xT columns on host (kills the ACT scale pass).
- PSUM budget: score tiles are per-head [128,512] (1 bank, bufs=3), PV
  accumulators [65,512] double-buffered (2+2 banks), outproj 1 bank. PV
  double-buffering removes the per-gp PE stall that kept HAM cold.
- reciprocal_approx_fast (~5x faster than reciprocal; 18 bits is plenty for
  bf16 probs), batched per-gp over both heads.
- wo prefetched on the scalar HWDGE queue during phase 1.
- softmax without max-subtraction: scores bounded, so
  attn = exp(s)/(sum exp(s) + exp(sink)) (== reference algebraically).
- scores computed transposed [sk, sq] so P^T feeds PV directly; PV output
  o^T [d, sq] feeds the out-proj directly. No S x S transposes anywhere.
"""

import math
import os
import sys
from collections import deque

sys.path.insert(0, "/opt/trn_rl_repo")

import numpy as np
import ml_dtypes

BF16 = ml_dtypes.bfloat16

# ---- problem constants (hardcoded per contract) ----
HID = 2880
S = 1536
N_HEADS = 64
N_KV = 8
D = 64
G = 8
SM_SCALE = 1.0 / math.sqrt(D)
EPS = 1e-5
NCORES = 8

ROPE_BASE = 150000.0
INIT_CTX = 4096
SCALING = 32.0
NTK_ALPHA = 1.0
NTK_BETA = 32.0

KP = 2944          # padded contraction dim: 2880 + bias row + zero pad = 23*128
KCH = KP // 128    # 23
QKV_O = 640        # 512 q + 64 k + 64 v per core
ETILES = (HID + 127) // 128  # 23 (22*128 + 64)


def _rope_tables(num_tokens: int):
    d_half = D // 2
    freq = ROPE_BASE ** (np.arange(0, D, 2, dtype=np.float64) / D)
    concentration = 0.1 * math.log(SCALING) + 1.0
    low = d_half * math.log(INIT_CTX / (NTK_BETA * 2 * math.pi)) / math.log(ROPE_BASE)
    high = d_half * math.log(INIT_CTX / (NTK_ALPHA * 2 * math.pi)) / math.log(ROPE_BASE)
    interpolation = 1.0 / (SCALING * freq)
    extrapolation = 1.0 / freq
    ramp = (np.arange(d_half, dtype=np.float64) - low) / (high - low)
    mask = 1.0 - np.clip(ramp, 0.0, 1.0)
    inv_freq = interpolation * (1.0 - mask) + extrapolation * mask
    t = np.arange(num_tokens, dtype=np.float64)
    freqs = np.outer(t, inv_freq)
    cos = (np.cos(freqs) * concentration).astype(np.float32)
    sin = (np.sin(freqs) * concentration).astype(np.float32)
    return cos, sin


_PROGRAM = None
LAST_EXEC_NS = None
LAST_RESULTS = None


def _build_program(s_len=S, reps=1):
    import concourse.bacc as bacc
    import concourse.tile as tile
    from concourse import mybir
    from contextlib import ExitStack

    f32 = mybir.dt.float32
    bf = mybir.dt.bfloat16
    Act = mybir.ActivationFunctionType

    stiles = s_len // 128
    sqc = s_len // 512

    nc = bacc.Bacc("TRN2", target_bir_lowering=False, debug=False)

    xT = nc.dram_tensor("xT", [KP, s_len], bf, kind="ExternalInput")
    wqkv = nc.dram_tensor("wqkv", [KP, QKV_O], bf, kind="ExternalInput")
    wo = nc.dram_tensor("wo", [512, HID], bf, kind="ExternalInput")
    cosd = nc.dram_tensor("cosd", [128, stiles * 32], bf, kind="ExternalInput")
    sind = nc.dram_tensor("sind", [128, stiles * 32], bf, kind="ExternalInput")
    sinkw = nc.dram_tensor("sinkw", [1, 8 * 65], bf, kind="ExternalInput")
    idend = nc.dram_tensor("idend", [128, 128], bf, kind="ExternalInput")
    maskd = nc.dram_tensor("maskd", [128, 4 * 512], bf, kind="ExternalInput")
    yT = nc.dram_tensor("yT", [HID, s_len], bf, kind="ExternalOutput")

    xT_r = xT[:].rearrange("(j p) s -> p j s", p=128)
    wqkv_r = wqkv[:].rearrange("(j p) o -> p j o", p=128)
    wo_r = wo[:].rearrange("(b p) e -> p b e", p=128)

    import concourse.bass as bass

    def bcast_mid(ap2d, n):
        a = ap2d
        return bass.AP(a.tensor, a.offset, [list(a.ap[0]), [0, n], list(a.ap[1])])

    with ExitStack() as top:
        tc = top.enter_context(tile.TileContext(nc))
        consts = top.enter_context(tc.tile_pool(name="consts", bufs=1))
        persist = top.enter_context(tc.tile_pool(name="persist", bufs=1))

        iden = consts.tile([128, 128], bf)
        nc.gpsimd.dma_start(out=iden[:], in_=idend[:])
        negm = consts.tile([128, 4, 512], bf)
        nc.gpsimd.dma_start(out=negm[:], in_=maskd[:].rearrange("p (l f) -> p l f", l=4))
        cos_t = consts.tile([128, stiles, 32], bf)
        nc.gpsimd.dma_start(out=cos_t[:], in_=cosd[:].rearrange("p (i d) -> p i d", d=32))
        sin_t = consts.tile([128, stiles, 32], bf)
        nc.gpsimd.dma_start(out=sin_t[:], in_=sind[:].rearrange("p (i d) -> p i d", d=32))
        sink_t = consts.tile([1, 8, 65], bf)
        nc.gpsimd.dma_start(out=sink_t[:], in_=sinkw[:].rearrange("p (g o) -> p g o", g=8))
        ones_row = consts.tile([1, 512], bf)
        nc.vector.memset(ones_row[:], 1.0)
        # dummy partition_broadcast: preloads the Q7 custom-op library during
        # phase 1 so the first softmax epilogue doesn't eat the LOAD_LIB stall
        gpw_in = consts.tile([1, 512], f32)
        nc.vector.memset(gpw_in[:], 1.0)
        gpw_out = consts.tile([64, 512], f32)
        nc.gpsimd.partition_broadcast(gpw_out[:], gpw_in[:], channels=64)

        qT2 = persist.tile([128, 8, s_len], bf)   # rows 0:64 = qT, 64:128 = dup
        kT2 = persist.tile([128, s_len], bf)
        vaug = persist.tile([128, stiles, 65], bf)
        nc.vector.memset(vaug[:, :, 64:65], 1.0)
        oT_s = persist.tile([128, 4, s_len], bf)
        wo_t = persist.tile([128, 4, HID], bf)

        for _rep in range(reps):
          # ---------------- phase 1-3: qkv proj + rope + transposes --------------
          with ExitStack() as ph1:
              wqp = ph1.enter_context(tc.tile_pool(name="wq", bufs=1))
              xsp = ph1.enter_context(tc.tile_pool(name="xs", bufs=2))
              qkvp = ph1.enter_context(tc.tile_pool(name="qkv", bufs=4))
              ropp = ph1.enter_context(tc.tile_pool(name="rop", bufs=4))
              tmpp = ph1.enter_context(tc.tile_pool(name="rtmp", bufs=3))
              p1a = ph1.enter_context(tc.tile_pool(name="p1a", bufs=2, space="PSUM"))
              p1b = ph1.enter_context(tc.tile_pool(name="p1b", bufs=2, space="PSUM"))
              ptr = ph1.enter_context(tc.tile_pool(name="ptr", bufs=3, space="PSUM"))

              # prefetch wo on the scalar HWDGE queue (parallel to sync queue)
              nc.scalar.dma_start(out=wo_t[:, :, 0:1440], in_=wo_r[:, :, 0:1440])
              nc.scalar.dma_start(out=wo_t[:, :, 1440:HID], in_=wo_r[:, :, 1440:HID])

              # first weight chunk, then first x block, then remaining weights
              wq_t = wqp.tile([128, KCH, QKV_O], bf)
              nc.sync.dma_start(out=wq_t[:, 0:1, :], in_=wqkv_r[:, 0:1, :])
              xt0 = xsp.tile([128, KCH, 256], bf, tag="xt")
              nc.sync.dma_start(out=xt0[:], in_=xT_r[:, :, 0:256])
              for a, b in ((1, 2), (2, 4), (4, 6), (6, 9), (9, 12),
                           (12, 16), (16, 20), (20, KCH)):
                  nc.sync.dma_start(out=wq_t[:, a:b, :], in_=wqkv_r[:, a:b, :])

              xt = xt0
              for i in range(stiles):
                  i2, sb = divmod(i, 2)
                  if sb == 0 and i2 > 0:
                      xt = xsp.tile([128, KCH, 256], bf, tag="xt")
                      nc.sync.dma_start(out=xt[:], in_=xT_r[:, :, i2 * 256:(i2 + 1) * 256])
                  xs2 = xt[:, :, sb * 128:(sb + 1) * 128]
                  pa = p1a.tile([128, 384], f32)
                  pb = p1b.tile([128, 256], f32)
                  for j in range(KCH):
                      nc.tensor.matmul(pa[:], xs2[:, j, :], wq_t[:, j, 0:384],
                                       start=(j == 0), stop=(j == KCH - 1))
                      nc.tensor.matmul(pb[:], xs2[:, j, :], wq_t[:, j, 384:640],
                                       start=(j == 0), stop=(j == KCH - 1))
                  qkvt = qkvp.tile([128, QKV_O], bf)
                  nc.scalar.activation(qkvt[:, 0:384], pa[:], Act.Copy)
                  nc.scalar.activation(qkvt[:, 384:640], pb[:], Act.Copy)

                  ro = ropp.tile([128, 576], bf)
                  # one strided view covers q heads 0-7 (stride 64) AND k (at 512)
                  q3 = qkvt[:, 0:576].rearrange("p (h d) -> p h d", h=9)
                  r3 = ro[:, 0:576].rearrange("p (h d) -> p h d", h=9)
                  ctb = bcast_mid(cos_t[:, i, :], 9)
                  stb = bcast_mid(sin_t[:, i, :], 9)
                  t1 = tmpp.tile([128, 9, 32], bf, tag="t1")
                  t2 = tmpp.tile([128, 9, 32], bf, tag="t2")
                  t3 = tmpp.tile([128, 9, 32], bf, tag="t3")
                  t4 = tmpp.tile([128, 9, 32], bf, tag="t4")
                  x1 = q3[:, :, 0:32]
                  x2 = q3[:, :, 32:64]
                  nc.vector.tensor_mul(t1[:], x1, ctb)
                  nc.vector.tensor_mul(t2[:], x2, stb)
                  nc.vector.tensor_sub(r3[:, :, 0:32], t1[:], t2[:])
                  nc.vector.tensor_mul(t3[:], x2, ctb)
                  nc.vector.tensor_mul(t4[:], x1, stb)
                  nc.vector.tensor_add(r3[:, :, 32:64], t3[:], t4[:])

                  nc.vector.tensor_copy(vaug[:, i, 0:64], qkvt[:, 576:640])

                  for gq in range(2):
                      ptq = ptr.tile([64, 512], bf, tag="pt")
                      for g4 in range(4):
                          g = gq * 4 + g4
                          nc.tensor.transpose(ptq[:, g4 * 128:(g4 + 1) * 128],
                                              ro[:, g * 64:(g + 1) * 64], iden[:])
                      nc.vector.tensor_copy(
                          qT2[0:64, gq * 4:(gq + 1) * 4, i * 128:(i + 1) * 128],
                          ptq[:].rearrange("p (h f) -> p h f", h=4))
                  ptk = ptr.tile([64, 512], bf, tag="pt")
                  nc.tensor.transpose(ptk[:, 0:128], ro[:, 512:576], iden[:])
                  nc.vector.tensor_copy(kT2[0:64, i * 128:(i + 1) * 128], ptk[:, 0:128])

                  if i % 4 == 3:
                      jc = i // 4
                      sq = slice(jc * 512, (jc + 1) * 512)
                      # duplicate q/k rows into partitions 64:128 for row-group packing
                      nc.sync.dma_start(out=qT2[64:128, :, sq], in_=qT2[0:64, :, sq])
                      nc.sync.dma_start(out=kT2[64:128, sq], in_=kT2[0:64, sq])

          # ---------------- phase 4+5: attention + out proj, interleaved ---------
          with ExitStack() as ph45:
              ptp = ph45.enter_context(tc.tile_pool(name="ptile", bufs=8))
              nrm = ph45.enter_context(tc.tile_pool(name="nrm", bufs=2))
              rbp = ph45.enter_context(tc.tile_pool(name="rbp", bufs=4))
              ytsp = ph45.enter_context(tc.tile_pool(name="yts", bufs=6))

              def make_emit(pool, nbufs):
                  def emit_outproj_block(jc_src, et, tail=False):
                      sqo = slice(jc_src * 512, (jc_src + 1) * 512)
                      esz = min(128, HID - et * 128)
                      es = slice(et * 128, et * 128 + esz)
                      ytp = pool.tile([128, 512], f32, tag="ytp", bufs=nbufs)
                      for b in range(4):
                          nc.tensor.matmul(ytp[0:esz, :], wo_t[:, b, es],
                                           oT_s[:, b, sqo],
                                           start=(b == 0), stop=(b == 3))
                      yts = ytsp.tile([128, 512], bf, tag="yts")
                      if tail:
                          nc.scalar.activation(yts[0:esz, :], ytp[0:esz, :], Act.Copy)
                      else:
                          nc.vector.tensor_copy(yts[0:esz, :], ytp[0:esz, :])
                      nc.sync.dma_start(out=yT[es, sqo], in_=yts[0:esz, :])
                  return emit_outproj_block

              pend = deque()
              attn = ExitStack()
              scp = attn.enter_context(tc.tile_pool(name="sc", bufs=3, space="PSUM"))
              pvp = attn.enter_context(tc.tile_pool(name="pv", bufs=2, space="PSUM"))
              ytpp = attn.enter_context(tc.tile_pool(name="ytp", bufs=1, space="PSUM"))
              emit_outproj_block = make_emit(ytpp, 1)
              # dense dummy matmuls in the phase-transition stall: flips the
              # HAM clock gate to 8/8 before the ACT-paced attention stream
              # (which alone has too low a PE duty cycle to warm it)
              for wi in range(16):
                  scw = scp.tile([128, 512], f32, tag="sc")
                  nc.tensor.matmul(scw[:], kT2[0:64, 0:128], qT2[0:64, 0, 0:512],
                                   start=True, stop=True, tile_position=(0, 0))
              for jc in range(sqc):
                  sq = slice(jc * 512, (jc + 1) * 512)
                  nsk = 4 * (jc + 1)
                  for gp in range(4):
                      g0, g1 = 2 * gp, 2 * gp + 1
                      pva = pvp.tile([65, 512], f32, tag="pva")
                      pvb = pvp.tile([65, 512], f32, tag="pvb")
                      nc.tensor.matmul(pva[:], sink_t[:, g0, :], ones_row[:],
                                       start=True, stop=False)
                      nc.tensor.matmul(pvb[:], sink_t[:, g1, :], ones_row[:],
                                       start=True, stop=False)
                      for isk in range(nsk):
                          ks = slice(isk * 128, (isk + 1) * 128)
                          lsi = isk - 4 * jc
                          sca = scp.tile([128, 512], f32, tag="sc")
                          scb = scp.tile([128, 512], f32, tag="sc")
                          nc.tensor.matmul(sca[:], kT2[0:64, ks], qT2[0:64, g0, sq],
                                           start=True, stop=True,
                                           tile_position=(0, 0))
                          nc.tensor.matmul(scb[:], kT2[64:128, ks], qT2[64:128, g1, sq],
                                           start=True, stop=True,
                                           tile_position=(64, 0))
                          pta = ptp.tile([128, 512], bf, tag="pt")
                          ptb = ptp.tile([128, 512], bf, tag="pt")
                          nc.scalar.activation(pta[:], sca[:], Act.Exp, scale=SM_SCALE)
                          nc.scalar.activation(ptb[:], scb[:], Act.Exp, scale=SM_SCALE)
                          if lsi >= 0:
                              w = 128 * (lsi + 1)  # mask is all-ones beyond col w
                              nc.vector.tensor_mul(pta[:, 0:w], pta[:, 0:w],
                                                   negm[:, lsi, 0:w])
                              nc.vector.tensor_mul(ptb[:, 0:w], ptb[:, 0:w],
                                                   negm[:, lsi, 0:w])
                          nc.tensor.matmul(pva[:], vaug[:, isk, :], pta[:],
                                           start=False, stop=(isk == nsk - 1))
                          nc.tensor.matmul(pvb[:], vaug[:, isk, :], ptb[:],
                                           start=False, stop=(isk == nsk - 1))
                          if pend:
                              emit_outproj_block(*pend.popleft())
                      # softmax epilogue: one reciprocal covers both heads
                      # (denoms staged at 32-aligned partitions; recip cost
                      # depends on free-size only, so [33,512] == [1,512])
                      dn = nrm.tile([33, 512], f32, tag="dn")
                      nc.vector.tensor_copy(dn[0:1, :], pva[64:65, :])
                      nc.vector.tensor_copy(dn[32:33, :], pvb[64:65, :])
                      rec = nrm.tile([33, 512], f32, tag="rec")
                      nc.vector.reciprocal(rec[:], dn[:])
                      # partition_broadcast (Q7 custom op) needs base-0 input
                      recb0 = nrm.tile([1, 512], f32, tag="recb0")
                      nc.vector.tensor_copy(recb0[:], rec[32:33, :])
                      rba = rbp.tile([64, 512], f32, tag="rb")
                      rbb = rbp.tile([64, 512], f32, tag="rb")
                      nc.gpsimd.partition_broadcast(rba[:], rec[0:1, :], channels=64)
                      nc.gpsimd.partition_broadcast(rbb[:], recb0[:], channels=64)
                      nc.vector.tensor_mul(oT_s[0:64, gp, sq], pva[0:64, :], rba[:])
                      # partition-shifted DVE writes are sim-only; stage + DMA
                      ot = rbp.tile([64, 512], bf, tag="ot")
                      nc.vector.tensor_mul(ot[:], pvb[0:64, :], rbb[:])
                      nc.sync.dma_start(out=oT_s[64:128, gp, sq], in_=ot[:])
                      if pend and jc < sqc - 1:
                          emit_outproj_block(*pend.popleft())
                  pend.extend((jc, et) for et in range(ETILES))
              # tail drain: free the attention PSUM banks and run the last
              # out-proj blocks through a deep 4-bank ring so they pipeline
              attn.close()
              with tc.tile_pool(name="ytail", bufs=4, space="PSUM") as ytailp:
                  emit_tail = make_emit(ytailp, 4)
                  while pend:
                      emit_tail(*pend.popleft(), tail=True)

    nc.finalize()
    return nc


def _get_program():
    global _PROGRAM
    if _PROGRAM is None:
        _PROGRAM = _build_program(S)
    return _PROGRAM


def _host_inputs(x, sinks, norm_scale, qkv_w, qkv_b, out_w, s_len=S):
    xf = np.ascontiguousarray(np.asarray(x, np.float32).reshape(s_len, HID))
    ms = np.mean(xf * xf, axis=1, dtype=np.float32)
    rnorm = (1.0 / np.sqrt(ms + np.float32(EPS))).astype(np.float32)
    cos, sin = _rope_tables(s_len)
    stiles = s_len // 128

    xTp = np.zeros((KP, s_len), BF16)
    xTp[:HID] = (xf.T * rnorm[None, :]).astype(BF16)
    xTp[HID] = BF16(1.0)  # bias row

    nsc = np.asarray(norm_scale, np.float32)
    qkvw = np.asarray(qkv_w, np.float32) * nsc[None, :]
    qkvb = np.asarray(qkv_b, np.float32)
    ow = np.asarray(out_w, np.float32)
    sk = np.asarray(sinks, np.float32)

    cos_t = np.ascontiguousarray(
        cos.reshape(stiles, 128, 32).transpose(1, 0, 2).reshape(128, stiles * 32)
    ).astype(BF16)
    sin_t = np.ascontiguousarray(
        sin.reshape(stiles, 128, 32).transpose(1, 0, 2).reshape(128, stiles * 32)
    ).astype(BF16)
    iden = np.eye(128, dtype=BF16)
    # msk[l][p,f] = 1 if valid (f >= 128*l + p) else 0; multiplies exp output
    pp = np.arange(128)[:, None]
    ff = np.arange(512)[None, :]
    masks = np.stack([(ff >= 128 * l + pp).astype(BF16) for l in range(4)], 1)
    masks = np.ascontiguousarray(masks.reshape(128, 4 * 512))

    in_maps = []
    for c in range(NCORES):
        heads = [g * 8 + c for g in range(G)]
        wq = np.concatenate([qkvw[h * 64:(h + 1) * 64] for h in heads], 0)
        wk = qkvw[4096 + c * 64:4096 + (c + 1) * 64]
        wv = qkvw[4608 + c * 64:4608 + (c + 1) * 64]
        wqkv_c = np.concatenate([wq, wk, wv], 0)          # [640, 2880]
        bq = np.concatenate([qkvb[h * 64:(h + 1) * 64] for h in heads]
                            + [qkvb[4096 + c * 64:4096 + (c + 1) * 64],
                               qkvb[4608 + c * 64:4608 + (c + 1) * 64]])
        wq_pad = np.zeros((KP, QKV_O), BF16)
        wq_pad[:HID] = wqkv_c.T.astype(BF16)
        wq_pad[HID] = bq.astype(BF16)
        cols = np.concatenate([np.arange(h * 64, (h + 1) * 64) for h in heads])
        woT = np.ascontiguousarray(ow[:, cols].T).astype(BF16)  # [512, 2880]
        sinkw = np.zeros((8, 65), BF16)
        for g in range(G):
            sinkw[g, 64] = BF16(np.exp(sk[heads[g]]))
        in_maps.append({
            "xT": xTp, "wqkv": wq_pad, "wo": woT,
            "cosd": cos_t, "sind": sin_t,
            "sinkw": sinkw.reshape(1, 8 * 65), "idend": iden, "maskd": masks,
        })
    return in_maps, xf


def kernel(x, sinks, norm_scale, qkv_w, qkv_b, out_w, out_b):
    global LAST_EXEC_NS, LAST_RESULTS
    from concourse.bass_utils import run_bass_kernel_spmd

    B = x.shape[0]
    in_maps, xf = _host_inputs(x, sinks, norm_scale, qkv_w, qkv_b, out_w)
    nc = _get_program()
    trace = bool(os.environ.get("KERNEL_TRACE"))
    if trace:
        try:
            from antenv.axon_hooks import get_axon_ntff_profile_hook  # noqa: F401
        except Exception:
            trace = False
    r = run_bass_kernel_spmd(nc, in_maps, core_ids=list(range(NCORES)), trace=trace)
    LAST_EXEC_NS = r.exec_time_ns
    LAST_RESULTS = r
    y = np.zeros((S, HID), np.float32)
    for c in range(NCORES):
        y += r.results[c]["yT"].T.astype(np.float32)
    out = xf + y + np.asarray(out_b, np.float32)[None, :]
    return out.reshape(B, S, HID).astype(np.float32)
